# revision 36
# baseline (speedup 1.0000x reference)
"""Trainium2 Bass kernel for LocalSparseAttention (anti-local windowed attention).

Reference computation (B=2, L=2048, D=512, H=8, hd=64):
    qkv = x @ in_proj_w.T + in_proj_b ; q,k,v = split(qkv)
    q *= 1/sqrt(hd)
    scores = q @ k.T  per head, with positions j in [i-w/2, i+w/2) BANNED (-inf)
    attn = softmax(scores); ctx = attn @ v
    out = LayerNorm(x + ctx @ out_proj_w.T + out_proj_b) * gamma + beta

Sharding: 8 cores = 2 batches x 4 query-shards of 512 rows. Each core
computes k/v for all 2048 keys of its batch (from a host-rotated x^T so
the banned diagonal band lands at fixed key-tile loop positions on every
core, keeping the SPMD graph uniform; masks are per-core 0/1 input data),
and full attention + out_proj + residual + LayerNorm for its 512 queries.

Math transformations (validated vs the reference):
  - k-bias dropped: softmax invariant.
  - v-bias folded into out_proj bias (attn rows sum to 1).
  - q scaled by 1/sqrt(hd) by scaling Wq/bq on host.
  - no max-subtraction in softmax (scores ~ N(0,1), exp safe); banned
    positions zeroed AFTER exp via 0/1 mask multiply.
  - softmax denominator via a ones-column appended to v (row 64 of the
    65-row ctx accumulator); divided out with a PE outer-product
    broadcast + DVE fast-reciprocal.

Datapath is bf16 (weights, x^T, k^T, v, q^T, exp(scores), masks, ctx)
with fp32 PSUM accumulation; residual x and LayerNorm stay fp32.  bf16
doubles PE streaming + LDWEIGHTS rate (FWL) vs fp32 and halves DMA.

Structure: 4 passes of (2 heads x 16 key tiles), each pass owning one
128-row chunk of q^T/k^T.  The softmax division of pass p overlaps pass
p+1's score/ctx matmuls (ping-pong PSUM ctx slots); k^T/v prep matmuls
are deadline-interleaved into the pass loops.
"""

import ml_dtypes
import numpy as np

import concourse.bass as bass
import concourse.tile as tile
import concourse.mybir as mybir
from concourse import bacc
from concourse.bass_utils import run_bass_kernel_spmd

F32 = mybir.dt.float32
BF16 = mybir.dt.bfloat16
AF = mybir.ActivationFunctionType
OP = mybir.AluOpType

B, L, D = 2, 2048, 512
H, HD = 8, 64
SH = L // 4            # 512-query shard per core
NJ = 16                # key tiles of 128 per sequence
MASK_SLOTS = [0, 1, 2, 3, 4, 15]   # key-tile loop positions that can carry the band
LN_EPS = 1e-5

USE_POOL_MASKS = False  # GpSimd ops need ucode libraries this stack lacks; keep DVE

_COMPILED = None
LAST_RESULT = None
STRIPS = []
LN_TRIVIAL = False


def _build(half, ln_trivial):
    global LN_TRIVIAL, STRIPS
    LN_TRIVIAL = ln_trivial
    STRIPS = []
    for j in MASK_SLOTS[:-1]:
        c0 = max(0, 128 * j - half + 1)
        c1 = min(SH, 128 * j + 128 + half)
        STRIPS.append((c0, max(c1, c0 + 1)))
    STRIPS.append((0, max(1, min(SH, half))))

    nc = bacc.Bacc("TRN2", target_bir_lowering=False, debug=False, num_devices=8)

    # All inputs are host-packed partition-major ([128, ...]) so each
    # logical load is ONE dma_start — the Sync queue serializes dma_start
    # instructions at ~0.6us each, so instruction count is what matters.
    strip_w = [c1 - c0 for c0, c1 in STRIPS]
    W_MASK = sum(strip_w)
    xT = nc.dram_tensor("xTp", [128, 4 * L], BF16, kind="ExternalInput")       # rotated x^T, d-chunk packed
    x_nat = nc.dram_tensor("xnatp", [128, 4 * D], BF16, kind="ExternalInput")  # query rows + folded out bias
    ident_d = nc.dram_tensor("ident", [128, 128], BF16, kind="ExternalInput")
    winT = nc.dram_tensor("winTp", [128, 4 * 3 * D], BF16, kind="ExternalInput")  # in_proj_w.T, q pre-scaled
    woutT = nc.dram_tensor("woutTp", [128, 4 * D], BF16, kind="ExternalInput")    # out_proj_w.T
    bq_d = nc.dram_tensor("bq", [128, 4], F32, kind="ExternalInput")           # scaled q bias, chunked
    masks_d = nc.dram_tensor("maskp", [128, W_MASK], BF16, kind="ExternalInput")
    if not ln_trivial:
        gamma_d = nc.dram_tensor("gamma", [128, D], F32, kind="ExternalInput")
        beta_d = nc.dram_tensor("beta", [128, D], F32, kind="ExternalInput")
    out_d = nc.dram_tensor("out", [SH, D], F32, kind="ExternalOutput")

    mask_engine = None  # resolved inside

    with tile.TileContext(nc) as tc:
        with (
            tc.tile_pool(name="persist", bufs=1) as pp,
            tc.tile_pool(name="work", bufs=2) as wp,
            tc.tile_pool(name="expp", bufs=4) as ep,
        ):
            mask_engine = nc.gpsimd if USE_POOL_MASKS else nc.vector

            # ---- DMA: one instruction per logical load; critical-path
            # bytes (x^T seg 0, q/k weight chunk 0, v weights) first ----
            bq_sb = pp.tile([128, 4], F32, tag="bq")
            nc.sync.dma_start(out=bq_sb, in_=bq_d[:, :])
            xT_all = pp.tile([128, 4 * L], BF16, tag="xT", name="xT_all")
            wq_all = pp.tile([128, 4 * D], BF16, tag="wq", name="wq_all")
            kv_all = pp.tile([128, 4 * 2 * D], BF16, tag="kv", name="kv_all")
            xT_r = xT_all.rearrange("p (d c) -> p d c", d=4)
            xTd_r = xT.rearrange("p (d c) -> p d c", d=4)
            wq_r = wq_all.rearrange("p (d c) -> p d c", d=4)
            kv_r = kv_all.rearrange("p (d c) -> p d c", d=4)
            win_r = winT.rearrange("p (d c) -> p d c", d=4)
            # x^T seg 0 (q^T + kt0 seg0 + first v preps)
            nc.sync.dma_start(out=xT_r[:, :, 0:512], in_=xTd_r[:, :, 0:512])
            # k-weight chunk 0 (longest chain: prep matmuls + cast before
            # the first scores), then q chunk 0, then v weights
            nc.sync.dma_start(out=kv_r[:, :, 0:128], in_=win_r[:, :, 512:640])
            nc.sync.dma_start(out=wq_r[:, :, 0:128], in_=win_r[:, :, 0:128])
            nc.sync.dma_start(out=kv_r[:, :, 512:1024], in_=win_r[:, :, 1024:1536])
            # masks (packed strips)
            mask_all = pp.tile([128, W_MASK], BF16, tag="maskp", name="mask_all")
            nc.sync.dma_start(out=mask_all, in_=masks_d[:, :])
            mask_sb = []
            moff = 0
            for i in range(len(MASK_SLOTS)):
                mask_sb.append(mask_all[:, moff:moff + strip_w[i]])
                moff += strip_w[i]
            # x^T segs 1-3, remaining q/k weight chunks
            nc.sync.dma_start(out=xT_r[:, :, 512:2048], in_=xTd_r[:, :, 512:2048])
            nc.sync.dma_start(out=wq_r[:, :, 128:512], in_=win_r[:, :, 128:512])
            nc.sync.dma_start(out=kv_r[:, :, 128:512], in_=win_r[:, :, 640:1024])
            wout_all = pp.tile([128, 4 * D], BF16, tag="woutp", name="wout_all")
            nc.sync.dma_start(out=wout_all, in_=woutT[:, :])
            woutT_sb = [wout_all[:, 512 * p:512 * p + 512] for p in range(4)]
            xnat_all = pp.tile([128, 4 * D], BF16, tag="xnatp", name="xnat_all")
            nc.sync.dma_start(out=xnat_all, in_=x_nat[:, :])
            x_nat_sb = [xnat_all[:, 512 * qt:512 * qt + 512] for qt in range(4)]
            ident_sb = pp.tile([128, 128], BF16, tag="ident")
            nc.sync.dma_start(out=ident_sb, in_=ident_d[:, :])
            if not ln_trivial:
                gamma_sb = pp.tile([128, D], F32, tag="gamma")
                nc.sync.dma_start(out=gamma_sb, in_=gamma_d[:, :])
                beta_sb = pp.tile([128, D], F32, tag="beta")
                nc.sync.dma_start(out=beta_sb, in_=beta_d[:, :])

            # ---- constants ----
            wupf = pp.tile([128, 256], F32, tag="wupf")
            nc.vector.memset(wupf, 0.001)
            wup = pp.tile([128, 256], BF16, tag="wup")
            nc.vector.tensor_copy(wup, wupf)
            ones_bf = pp.tile([1, 64], BF16, tag="ones1")
            nc.vector.memset(ones_bf, 1.0)
            ones8 = pp.tile([128, 8], BF16, tag="ones8")
            nc.vector.memset(ones8, 1.0)
            eps_t = pp.tile([128, 1], F32, tag="eps")
            nc.vector.memset(eps_t, LN_EPS)

            ctxTs_sb = [pp.tile([128, SH], BF16, tag=f"ctxTs{p}", name=f"ctxTs{p}") for p in range(4)]
            kt_sb = [pp.tile([128, L], BF16, tag=f"kt{c}", name=f"kt{c}") for c in range(4)]
            v_sb = [pp.tile([128, H * (HD + 1)], BF16, tag=f"v{l2}", name=f"v{l2}") for l2 in range(NJ)]

            # PE warm-up: keep the PE busy (and the HAM clock-gate window
            # filling) while the first input DMAs land.
            with tc.tile_pool(name="wups", bufs=1, space="PSUM") as wps:
                wu_ps = wps.tile([128, 256], F32, tag="wu")
                for i in range(16):
                    nc.tensor.matmul(
                        wu_ps, wup[:, 0:128], wup,
                        start=(i == 0), stop=(i == 15),
                    )

            with tc.tile_pool(name="cxp", bufs=2, space="PSUM") as cxp:
              with tc.tile_pool(name="scp", bufs=2, space="PSUM") as scp:
                # ---- q^T: [D, SH] as 4 chunks of [128, SH]; chunk c is
                # only needed by pass c, so chunks 1-3 are emitted late ----
                qT_sb = [None] * 4

                def emit_qT(c):
                    ps = scp.tile([128, SH], F32, tag="sc", name=f"qps{c}")
                    for d in range(4):
                        nc.tensor.matmul(
                            ps,
                            wq_all[:, 512 * d + 128 * c:512 * d + 128 * c + 128],
                            xT_all[:, 2048 * d:2048 * d + SH],
                            start=(d == 0), stop=(d == 3),
                        )
                    qt = pp.tile([128, SH], BF16, tag=f"qT{c}", name=f"qT{c}")
                    nc.vector.tensor_scalar_add(qt, ps, bq_sb[:, c:c + 1])
                    qT_sb[c] = qt

                def emit_kt(c2, seg):
                    ps = scp.tile([128, 512], F32, tag="sc", name=f"ktps{c2}_{seg}")
                    for d in range(4):
                        nc.tensor.matmul(
                            ps,
                            kv_all[:, 1024 * d + 128 * c2:1024 * d + 128 * c2 + 128],
                            xT_all[:, 2048 * d + 512 * seg:2048 * d + 512 * seg + 512],
                            start=(d == 0), stop=(d == 3),
                        )
                    nc.vector.tensor_copy(kt_sb[c2][:, 512 * seg:512 * seg + 512], ps)

                def emit_v(l2):
                    ps = scp.tile([128, 512], F32, tag="sc", name=f"vps{l2}")
                    for d in range(4):
                        nc.tensor.matmul(
                            ps,
                            xT_all[:, 2048 * d + 128 * l2:2048 * d + 128 * l2 + 128],
                            kv_all[:, 1024 * d + 512:1024 * d + 1024],
                            start=(d == 0), stop=(d == 3),
                        )
                    vr = v_sb[l2].rearrange("p (t c) -> p t c", c=HD + 1)
                    nc.vector.tensor_copy(
                        vr[:, :, HD:HD + 1],
                        ones8.rearrange("p (t c) -> p t c", c=1),
                    )
                    nc.vector.tensor_copy(
                        vr[:, :, 0:HD],
                        ps.rearrange("p (t c) -> p t c", c=HD),
                    )

                # prep deadlines: v[l2] first used at pass 0 iter l2; kt
                # chunk c seg s first used at pass c iter 4s; qT chunk c at
                # pass c iter 0.  Keep the pre-pass minimal so the first
                # exp fires early, and level the rest into the pass loops.
                emit_kt(0, 0)
                emit_qT(0)
                prep_at = {}
                for l2 in range(NJ):
                    prep_at.setdefault((0, max(0, l2 - 1)), []).append(("v", l2, None))
                for s in range(1, 4):
                    prep_at.setdefault((0, 4 * s - 3), []).append(("kt", 0, s))
                for c in range(1, 4):
                    prep_at.setdefault((c - 1, 12), []).append(("qt", c, None))
                    prep_at.setdefault((c - 1, 13), []).append(("kt", c, 0))
                    for s in range(1, 4):
                        prep_at.setdefault((c, 4 * s - 3), []).append(("kt", c, s))

                # softmax division: Z-row cast to SBUF (DVE), PE outer-product
                # broadcast, fast-reciprocal + multiply (DVE).  The PE ops of
                # pass p's division are deferred until pass p+1's iter 1 so
                # the in-order PE queue never stalls on the Z cast.
                def div_stage1(p, ctx_pair):
                    # per-half casts so the stage-2 chain pipelines by halves
                    zbf = wp.tile([1, 2 * SH], BF16, tag="zbf", name=f"zbf{p}")
                    for t in range(2):
                        nc.vector.tensor_copy(
                            zbf[0:1, 512 * t:512 * t + 512],
                            ctx_pair[HD:HD + 1, 512 * t:512 * t + 512],
                        )
                    return zbf

                def div_stage2(p, ctx_pair, zbf, pool, pool_tag):
                    bc = pool.tile([64, 2 * SH], F32, tag=pool_tag, name=f"bc{p}")
                    rb = wp.tile([64, 2 * SH], F32, tag="rb", name=f"rb{p}")
                    for t in range(2):
                        nc.tensor.matmul(
                            bc[:, 512 * t:512 * t + 512],
                            ones_bf[0:1, 0:HD],
                            zbf[0:1, 512 * t:512 * t + 512],
                            start=True, stop=True,
                        )
                    for t in range(2):
                        nc.vector.reciprocal_approx_fast(
                            rb[:, 512 * t:512 * t + 512], bc[:, 512 * t:512 * t + 512]
                        )
                        nc.vector.tensor_tensor(
                            out=ctxTs_sb[p][64 * t:64 * t + 64, :],
                            in0=ctx_pair[0:HD, 512 * t:512 * t + 512],
                            in1=rb[:, 512 * t:512 * t + 512],
                            op=OP.mult,
                        )

                pending_div = None

                for p in range(4):
                    ctx_pair = cxp.tile([HD + 1, 2 * SH], F32, tag="ctx", name=f"ctx{p}")
                    for j in range(NJ):
                        sc = scp.tile([128, 2 * SH], F32, tag="sc")
                        for t in range(2):
                            nc.tensor.matmul(
                                sc[:, SH * t:SH * t + SH],
                                kt_sb[p][64 * t:64 * t + 64, 128 * j:128 * j + 128],
                                qT_sb[p][64 * t:64 * t + 64, :],
                                start=True, stop=True,
                            )
                        e = ep.tile([128, 2 * SH], BF16, tag="exp")
                        nc.scalar.activation(e, sc, AF.Exp)
                        if j in MASK_SLOTS:
                            slot = MASK_SLOTS.index(j)
                            c0, c1 = STRIPS[slot]
                            for t in range(2):
                                mask_engine.tensor_tensor(
                                    out=e[:, SH * t + c0:SH * t + c1],
                                    in0=e[:, SH * t + c0:SH * t + c1],
                                    in1=mask_sb[slot],
                                    op=OP.mult,
                                )
                        # prep matmuls AFTER this iter's scores: they fill
                        # the PE while exp runs instead of delaying it
                        for kind, a, b in prep_at.get((p, j), []):
                            if kind == "kt":
                                emit_kt(a, b)
                            elif kind == "qt":
                                emit_qT(a)
                            else:
                                emit_v(a)
                        for t in range(2):
                            h = 2 * p + t
                            nc.tensor.matmul(
                                ctx_pair[:, SH * t:SH * t + SH],
                                v_sb[j][:, (HD + 1) * h:(HD + 1) * h + HD + 1],
                                e[:, SH * t:SH * t + SH],
                                start=(j == 0), stop=(j == NJ - 1),
                            )
                        if j == 1 and pending_div is not None:
                            div_stage2(*pending_div, scp, "sc")
                            pending_div = None
                    zbf = div_stage1(p, ctx_pair)
                    if p < 3:
                        pending_div = (p, ctx_pair, zbf)
                    else:
                        last_div = (p, ctx_pair, zbf)

              # ---- out_proj + residual + LayerNorm per query tile ----
              # (scp is closed here; cxp stays open so pass 3's division can
              # finish while the p=0..2 out_proj partials run on freed scp
              # banks.)  The residual rides the PSUM accumulation (identity
              # matmul on x_nat); the final (y - mu) * rstd runs on the
              # scalar engine (Identity activation with per-row scale/bias).
              with tc.tile_pool(name="ops", bufs=1, space="PSUM") as ops:
                po_t = [ops.tile([128, D], F32, tag=f"po{qt}", name=f"po{qt}") for qt in range(4)]
                for qt in range(4):
                    nc.tensor.matmul(
                        po_t[qt], ident_sb, x_nat_sb[qt],
                        start=True, stop=False,
                    )
                for pp_i in range(3):
                    for qt in range(4):
                        nc.tensor.matmul(
                            po_t[qt],
                            ctxTs_sb[pp_i][:, 128 * qt:128 * qt + 128],
                            woutT_sb[pp_i],
                            start=False, stop=False,
                        )
                div_stage2(*last_div, cxp, "ctx")
                for qt in range(4):
                    po = po_t[qt]
                    nc.tensor.matmul(
                        po,
                        ctxTs_sb[3][:, 128 * qt:128 * qt + 128],
                        woutT_sb[3],
                        start=False, stop=True,
                    )
                    stats = wp.tile([128, 6], F32, tag="stats")
                    nc.vector.bn_stats(stats, po)
                    mv = wp.tile([128, 2], F32, tag="mv")
                    nc.vector.bn_aggr(mv, stats)
                    veps = wp.tile([128, 1], F32, tag="veps")
                    nc.vector.tensor_scalar_add(veps, mv[:, 1:2], eps_t)
                    rvar = wp.tile([128, 1], F32, tag="rvar")
                    nc.vector.reciprocal_approx_fast(rvar, veps)
                    rstd = wp.tile([128, 1], F32, tag="rstd")
                    nc.scalar.activation(rstd, rvar, AF.Sqrt)
                    nbias = wp.tile([128, 1], F32, tag="nbias")
                    nc.vector.tensor_scalar(
                        out=nbias, in0=mv[:, 0:1], scalar1=rstd, scalar2=-1.0,
                        op0=OP.mult, op1=OP.mult,
                    )
                    t1 = wp.tile([128, D], F32, tag="t1")
                    nc.scalar.activation(t1, po, AF.Identity, bias=nbias, scale=rstd)
                    if not ln_trivial:
                        nc.vector.tensor_tensor(out=t1, in0=t1, in1=gamma_sb, op=OP.mult)
                        nc.vector.tensor_tensor(out=t1, in0=t1, in1=beta_sb, op=OP.add)
                    nc.sync.dma_start(out=out_d[128 * qt:128 * qt + 128, :], in_=t1)

    nc.compile()
    return nc


def _host_prep(x, in_proj_w, in_proj_b, out_proj_w, out_proj_b, ln_gamma, ln_beta, window_size):
    x = np.ascontiguousarray(np.asarray(x, dtype=np.float32))
    in_proj_w = np.asarray(in_proj_w, dtype=np.float32)
    in_proj_b = np.asarray(in_proj_b, dtype=np.float32)
    out_proj_w = np.asarray(out_proj_w, dtype=np.float32)
    out_proj_b = np.asarray(out_proj_b, dtype=np.float32)
    ln_gamma = np.asarray(ln_gamma, dtype=np.float32)
    ln_beta = np.asarray(ln_beta, dtype=np.float32)
    w = int(np.asarray(window_size))
    half = w // 2
    assert half <= 128, "mask slots only cover |k-q| <= 128"

    bf16 = ml_dtypes.bfloat16

    def dpack(a):  # [4*128, C] -> [128, 4*C] partition-major d-chunk packing
        r, cc = a.shape
        return np.ascontiguousarray(
            a.reshape(4, 128, cc).transpose(1, 0, 2).reshape(128, 4 * cc)
        )

    scale = np.float32(1.0 / np.sqrt(HD))
    W = in_proj_w.copy()
    W[0:D] *= scale
    winT = dpack(np.ascontiguousarray(W.T)).astype(bf16)            # [128, 4*3D]
    woutT = dpack(np.ascontiguousarray(out_proj_w.T)).astype(bf16)  # [128, 4*D]
    bq = np.ascontiguousarray((in_proj_b[0:D] * scale).reshape(4, 128).T)  # [128, 4]
    bout = (out_proj_b + out_proj_w @ in_proj_b[2 * D:3 * D]).reshape(1, D)
    gamma_b = np.ascontiguousarray(np.broadcast_to(ln_gamma, (128, D)))
    beta_b = np.ascontiguousarray(np.broadcast_to(ln_beta, (128, D)))
    strips = []
    for j in MASK_SLOTS[:-1]:
        c0 = max(0, 128 * j - half + 1)
        c1 = min(SH, 128 * j + 128 + half)
        strips.append((c0, max(c1, c0 + 1)))
    strips.append((0, max(1, min(SH, half))))

    in_maps = []
    for c in range(8):
        b, s = divmod(c, 4)
        rot = (SH * s + np.arange(L)) % L
        xT_rot = dpack(np.ascontiguousarray(x[b][rot].T)).astype(bf16)  # [128, 4*L]
        x_nat = dpack(np.ascontiguousarray(
            x[b][SH * s:SH * s + SH] + bout[None, 0, :]
        )).astype(bf16)  # [128, 4*D]
        q_true = SH * s + np.arange(SH)[None, :]
        mask_cols = []
        for i, j in enumerate(MASK_SLOTS):
            k_true = (SH * s + 128 * j + np.arange(128)[:, None]) % L
            dd = k_true - q_true
            banned = (dd >= -half) & (dd < half)
            c0, c1 = strips[i]
            mask_cols.append((1.0 - banned[:, c0:c1].astype(np.float32)))
        maskp = np.concatenate(mask_cols, axis=1)
        m = {
            "xTp": xT_rot, "xnatp": x_nat, "winTp": winT, "woutTp": woutT,
            "bq": bq, "maskp": maskp.astype(bf16),
            "ident": np.eye(128, dtype=np.float32).astype(bf16),
        }
        if not LN_TRIVIAL:
            m["gamma"] = gamma_b
            m["beta"] = beta_b
        in_maps.append(m)
    return in_maps


def kernel(x, in_proj_w, in_proj_b, out_proj_w, out_proj_b, ln_gamma, ln_beta, window_size):
    global _COMPILED, LAST_RESULT
    half = int(np.asarray(window_size)) // 2
    ln_trivial = bool(np.all(np.asarray(ln_gamma) == 1.0) and np.all(np.asarray(ln_beta) == 0.0))
    key = (half, ln_trivial)
    if _COMPILED is None or _COMPILED[0] != key:
        _COMPILED = (key, _build(half, ln_trivial))
    in_maps = _host_prep(x, in_proj_w, in_proj_b, out_proj_w, out_proj_b,
                         ln_gamma, ln_beta, window_size)
    res = run_bass_kernel_spmd(_COMPILED[1], in_maps, core_ids=list(range(8)))
    LAST_RESULT = res
    out = np.empty((B, L, D), np.float32)
    for c in range(8):
        b, s = divmod(c, 4)
        out[b, SH * s:SH * s + SH] = res.results[c]["out"]
    return out


# revision 40
# speedup vs baseline: 1.0187x; 1.0187x over previous
"""Trainium2 Bass kernel for LocalSparseAttention (anti-local windowed attention).

Reference computation (B=2, L=2048, D=512, H=8, hd=64):
    qkv = x @ in_proj_w.T + in_proj_b ; q,k,v = split(qkv)
    q *= 1/sqrt(hd)
    scores = q @ k.T  per head, with positions j in [i-w/2, i+w/2) BANNED (-inf)
    attn = softmax(scores); ctx = attn @ v
    out = LayerNorm(x + ctx @ out_proj_w.T + out_proj_b) * gamma + beta

Sharding: 8 cores = 2 batches x 4 query-shards of 512 rows. Each core
computes k/v for all 2048 keys of its batch (from a host-rotated x^T so
the banned diagonal band lands at fixed key-tile loop positions on every
core, keeping the SPMD graph uniform; masks are per-core 0/1 input data),
and full attention + out_proj + residual + LayerNorm for its 512 queries.

Math transformations (validated vs the reference):
  - k-bias dropped: softmax invariant.
  - v-bias folded into out_proj bias (attn rows sum to 1).
  - q scaled by 1/sqrt(hd) by scaling Wq/bq on host.
  - no max-subtraction in softmax (scores ~ N(0,1), exp safe); banned
    positions zeroed AFTER exp via 0/1 mask multiply.
  - softmax denominator via a ones-column appended to v (row 64 of the
    65-row ctx accumulator); divided out with a PE outer-product
    broadcast + DVE fast-reciprocal.

Datapath is bf16 (weights, x^T, k^T, v, q^T, exp(scores), masks, ctx)
with fp32 PSUM accumulation; residual x and LayerNorm stay fp32.  bf16
doubles PE streaming + LDWEIGHTS rate (FWL) vs fp32 and halves DMA.

Structure: 4 passes of (2 heads x 16 key tiles), each pass owning one
128-row chunk of q^T/k^T.  The softmax division of pass p overlaps pass
p+1's score/ctx matmuls (ping-pong PSUM ctx slots); k^T/v prep matmuls
are deadline-interleaved into the pass loops.
"""

import ml_dtypes
import numpy as np

import concourse.bass as bass
import concourse.tile as tile
import concourse.mybir as mybir
from concourse import bacc
from concourse.bass_utils import run_bass_kernel_spmd

F32 = mybir.dt.float32
BF16 = mybir.dt.bfloat16
AF = mybir.ActivationFunctionType
OP = mybir.AluOpType

B, L, D = 2, 2048, 512
H, HD = 8, 64
SH = L // 4            # 512-query shard per core
NJ = 16                # key tiles of 128 per sequence
MASK_SLOTS = [0, 1, 2, 3, 4, 15]   # key-tile loop positions that can carry the band
LN_EPS = 1e-5

USE_POOL_MASKS = False  # GpSimd ops need ucode libraries this stack lacks; keep DVE

_COMPILED = None
LAST_RESULT = None
STRIPS = []
LN_TRIVIAL = False


def _build(half, ln_trivial):
    global LN_TRIVIAL, STRIPS
    LN_TRIVIAL = ln_trivial
    STRIPS = []
    for j in MASK_SLOTS[:-1]:
        c0 = max(0, 128 * j - half + 1)
        c1 = min(SH, 128 * j + 128 + half)
        STRIPS.append((c0, max(c1, c0 + 1)))
    STRIPS.append((0, max(1, min(SH, half))))

    nc = bacc.Bacc("TRN2", target_bir_lowering=False, debug=False, num_devices=8)

    # All inputs are host-packed partition-major ([128, ...]) so each
    # logical load is ONE dma_start — the Sync queue serializes dma_start
    # instructions at ~0.6us each, so instruction count is what matters.
    strip_w = [c1 - c0 for c0, c1 in STRIPS]
    W_MASK = sum(strip_w)
    xT = nc.dram_tensor("xTp", [128, 4 * L], BF16, kind="ExternalInput")       # rotated x^T, d-chunk packed
    x_nat = nc.dram_tensor("xnatp", [128, 4 * D], BF16, kind="ExternalInput")  # query rows + folded out bias
    ident_d = nc.dram_tensor("ident", [128, 128], BF16, kind="ExternalInput")
    winT = nc.dram_tensor("winTp", [128, 4 * 3 * D], BF16, kind="ExternalInput")  # in_proj_w.T, q pre-scaled
    woutT = nc.dram_tensor("woutTp", [128, 4 * D], BF16, kind="ExternalInput")    # out_proj_w.T
    bq_d = nc.dram_tensor("bq", [128, 4], F32, kind="ExternalInput")           # scaled q bias, chunked
    masks_d = nc.dram_tensor("maskp", [128, W_MASK], BF16, kind="ExternalInput")
    if not ln_trivial:
        gamma_d = nc.dram_tensor("gamma", [128, D], F32, kind="ExternalInput")
        beta_d = nc.dram_tensor("beta", [128, D], F32, kind="ExternalInput")
    out_d = nc.dram_tensor("out", [SH, D], F32, kind="ExternalOutput")

    mask_engine = None  # resolved inside

    with tile.TileContext(nc) as tc:
        with (
            tc.tile_pool(name="persist", bufs=1) as pp,
            tc.tile_pool(name="work", bufs=2) as wp,
            tc.tile_pool(name="expp", bufs=4) as ep,
        ):
            mask_engine = nc.gpsimd if USE_POOL_MASKS else nc.vector

            # ---- DMA: one instruction per logical load; critical-path
            # bytes (x^T seg 0, q/k weight chunk 0, v weights) first ----
            bq_sb = pp.tile([128, 4], F32, tag="bq")
            nc.sync.dma_start(out=bq_sb, in_=bq_d[:, :])
            xT_all = pp.tile([128, 4 * L], BF16, tag="xT", name="xT_all")
            wq_all = pp.tile([128, 4 * D], BF16, tag="wq", name="wq_all")
            kv_all = pp.tile([128, 4 * 2 * D], BF16, tag="kv", name="kv_all")
            xT_r = xT_all.rearrange("p (d c) -> p d c", d=4)
            xTd_r = xT.rearrange("p (d c) -> p d c", d=4)
            wq_r = wq_all.rearrange("p (d c) -> p d c", d=4)
            kv_r = kv_all.rearrange("p (d c) -> p d c", d=4)
            win_r = winT.rearrange("p (d c) -> p d c", d=4)
            # x^T seg 0 (q^T + kt0 seg0 + first v preps)
            nc.sync.dma_start(out=xT_r[:, :, 0:512], in_=xTd_r[:, :, 0:512])
            # q-weight chunk 0, k-weight chunk 0, v weights
            nc.sync.dma_start(out=wq_r[:, :, 0:128], in_=win_r[:, :, 0:128])
            nc.sync.dma_start(out=kv_r[:, :, 0:128], in_=win_r[:, :, 512:640])
            nc.sync.dma_start(out=kv_r[:, :, 512:1024], in_=win_r[:, :, 1024:1536])
            # masks (packed strips)
            mask_all = pp.tile([128, W_MASK], BF16, tag="maskp", name="mask_all")
            nc.sync.dma_start(out=mask_all, in_=masks_d[:, :])
            mask_sb = []
            moff = 0
            for i in range(len(MASK_SLOTS)):
                mask_sb.append(mask_all[:, moff:moff + strip_w[i]])
                moff += strip_w[i]
            # x^T segs 1-3, remaining q/k weight chunks
            nc.sync.dma_start(out=xT_r[:, :, 512:2048], in_=xTd_r[:, :, 512:2048])
            nc.sync.dma_start(out=wq_r[:, :, 128:512], in_=win_r[:, :, 128:512])
            nc.sync.dma_start(out=kv_r[:, :, 128:512], in_=win_r[:, :, 640:1024])
            wout_all = pp.tile([128, 4 * D], BF16, tag="woutp", name="wout_all")
            nc.sync.dma_start(out=wout_all, in_=woutT[:, :])
            woutT_sb = [wout_all[:, 512 * p:512 * p + 512] for p in range(4)]
            xnat_all = pp.tile([128, 4 * D], BF16, tag="xnatp", name="xnat_all")
            nc.sync.dma_start(out=xnat_all, in_=x_nat[:, :])
            x_nat_sb = [xnat_all[:, 512 * qt:512 * qt + 512] for qt in range(4)]
            ident_sb = pp.tile([128, 128], BF16, tag="ident")
            nc.sync.dma_start(out=ident_sb, in_=ident_d[:, :])
            if not ln_trivial:
                gamma_sb = pp.tile([128, D], F32, tag="gamma")
                nc.sync.dma_start(out=gamma_sb, in_=gamma_d[:, :])
                beta_sb = pp.tile([128, D], F32, tag="beta")
                nc.sync.dma_start(out=beta_sb, in_=beta_d[:, :])

            # ---- constants ----
            wupf = pp.tile([128, 256], F32, tag="wupf")
            nc.vector.memset(wupf, 0.001)
            wup = pp.tile([128, 256], BF16, tag="wup")
            nc.vector.tensor_copy(wup, wupf)
            ones_bf = pp.tile([1, 64], BF16, tag="ones1")
            nc.vector.memset(ones_bf, 1.0)
            ones8 = pp.tile([128, 8], BF16, tag="ones8")
            nc.vector.memset(ones8, 1.0)
            eps_t = pp.tile([128, 1], F32, tag="eps")
            nc.vector.memset(eps_t, LN_EPS)

            ctxTs_sb = [pp.tile([128, SH], BF16, tag=f"ctxTs{p}", name=f"ctxTs{p}") for p in range(4)]
            kt_sb = [pp.tile([128, L], BF16, tag=f"kt{c}", name=f"kt{c}") for c in range(4)]
            v_sb = [pp.tile([128, H * (HD + 1)], BF16, tag=f"v{l2}", name=f"v{l2}") for l2 in range(NJ)]

            # PE warm-up: keep the PE busy (and the HAM clock-gate window
            # filling) while the first input DMAs land.
            with tc.tile_pool(name="wups", bufs=1, space="PSUM") as wps:
                wu_ps = wps.tile([128, 256], F32, tag="wu")
                for i in range(12):
                    nc.tensor.matmul(
                        wu_ps, wup[:, 0:128], wup,
                        start=(i == 0), stop=(i == 11),
                    )

            with tc.tile_pool(name="cxp", bufs=2, space="PSUM") as cxp:
              with tc.tile_pool(name="scp", bufs=2, space="PSUM") as scp:
                # ---- q^T: [D, SH] as 4 chunks of [128, SH]; chunk c is
                # only needed by pass c, so chunks 1-3 are emitted late ----
                qT_sb = [None] * 4

                def emit_qT(c):
                    ps = scp.tile([128, SH], F32, tag="sc", name=f"qps{c}")
                    for d in range(4):
                        nc.tensor.matmul(
                            ps,
                            wq_all[:, 512 * d + 128 * c:512 * d + 128 * c + 128],
                            xT_all[:, 2048 * d:2048 * d + SH],
                            start=(d == 0), stop=(d == 3),
                        )
                    qt = pp.tile([128, SH], BF16, tag=f"qT{c}", name=f"qT{c}")
                    nc.vector.tensor_scalar_add(qt, ps, bq_sb[:, c:c + 1])
                    qT_sb[c] = qt

                def emit_kt(c2, seg):
                    ps = scp.tile([128, 512], F32, tag="sc", name=f"ktps{c2}_{seg}")
                    for d in range(4):
                        nc.tensor.matmul(
                            ps,
                            kv_all[:, 1024 * d + 128 * c2:1024 * d + 128 * c2 + 128],
                            xT_all[:, 2048 * d + 512 * seg:2048 * d + 512 * seg + 512],
                            start=(d == 0), stop=(d == 3),
                        )
                    nc.vector.tensor_copy(kt_sb[c2][:, 512 * seg:512 * seg + 512], ps)

                def emit_v(l2):
                    ps = scp.tile([128, 512], F32, tag="sc", name=f"vps{l2}")
                    for d in range(4):
                        nc.tensor.matmul(
                            ps,
                            xT_all[:, 2048 * d + 128 * l2:2048 * d + 128 * l2 + 128],
                            kv_all[:, 1024 * d + 512:1024 * d + 1024],
                            start=(d == 0), stop=(d == 3),
                        )
                    vr = v_sb[l2].rearrange("p (t c) -> p t c", c=HD + 1)
                    nc.vector.tensor_copy(
                        vr[:, :, HD:HD + 1],
                        ones8.rearrange("p (t c) -> p t c", c=1),
                    )
                    nc.vector.tensor_copy(
                        vr[:, :, 0:HD],
                        ps.rearrange("p (t c) -> p t c", c=HD),
                    )

                # prep deadlines: v[l2] first used at pass 0 iter l2; kt
                # chunk c seg s first used at pass c iter 4s; qT chunk c at
                # pass c iter 0.  Keep the pre-pass minimal so the first
                # exp fires early, and level the rest into the pass loops.
                emit_qT(0)
                emit_kt(0, 0)
                prep_at = {}
                for l2 in range(NJ):
                    prep_at.setdefault((0, max(0, l2 - 2)), []).append(("v", l2, None))
                for s in range(1, 4):
                    prep_at.setdefault((0, 4 * s - 3), []).append(("kt", 0, s))
                for c in range(1, 4):
                    prep_at.setdefault((c - 1, 12), []).append(("qt", c, None))
                    prep_at.setdefault((c - 1, 13), []).append(("kt", c, 0))
                    for s in range(1, 4):
                        prep_at.setdefault((c, 4 * s - 3), []).append(("kt", c, s))

                # softmax division: Z-row cast to SBUF (DVE), PE outer-product
                # broadcast, fast-reciprocal + multiply (DVE).  The PE ops of
                # pass p's division are deferred until pass p+1's iter 1 so
                # the in-order PE queue never stalls on the Z cast.
                def div_stage1(p, ctx_pair):
                    zbf = wp.tile([1, 2 * SH], BF16, tag="zbf", name=f"zbf{p}")
                    nc.vector.tensor_copy(zbf, ctx_pair[HD:HD + 1, :])
                    return zbf

                def div_stage2(p, ctx_pair, zbf, pool, pool_tag):
                    bc = pool.tile([64, 2 * SH], F32, tag=pool_tag, name=f"bc{p}")
                    for t in range(2):
                        nc.tensor.matmul(
                            bc[:, 512 * t:512 * t + 512],
                            ones_bf[0:1, 0:HD],
                            zbf[0:1, 512 * t:512 * t + 512],
                            start=True, stop=True,
                        )
                    rb = wp.tile([64, 2 * SH], F32, tag="rb", name=f"rb{p}")
                    nc.vector.reciprocal_approx_fast(rb, bc)
                    for t in range(2):
                        nc.vector.tensor_tensor(
                            out=ctxTs_sb[p][64 * t:64 * t + 64, :],
                            in0=ctx_pair[0:HD, 512 * t:512 * t + 512],
                            in1=rb[:, 512 * t:512 * t + 512],
                            op=OP.mult,
                        )

                pending_div = None

                for p in range(4):
                    ctx_pair = cxp.tile([HD + 1, 2 * SH], F32, tag="ctx", name=f"ctx{p}")
                    for j in range(NJ):
                        sc = scp.tile([128, 2 * SH], F32, tag="sc")
                        for t in range(2):
                            nc.tensor.matmul(
                                sc[:, SH * t:SH * t + SH],
                                kt_sb[p][64 * t:64 * t + 64, 128 * j:128 * j + 128],
                                qT_sb[p][64 * t:64 * t + 64, :],
                                start=True, stop=True,
                            )
                        e = ep.tile([128, 2 * SH], BF16, tag="exp")
                        nc.scalar.activation(e, sc, AF.Exp)
                        if j in MASK_SLOTS:
                            slot = MASK_SLOTS.index(j)
                            c0, c1 = STRIPS[slot]
                            for t in range(2):
                                mask_engine.tensor_tensor(
                                    out=e[:, SH * t + c0:SH * t + c1],
                                    in0=e[:, SH * t + c0:SH * t + c1],
                                    in1=mask_sb[slot],
                                    op=OP.mult,
                                )
                        # prep matmuls AFTER this iter's scores: they fill
                        # the PE while exp runs instead of delaying it
                        for kind, a, b in prep_at.get((p, j), []):
                            if kind == "kt":
                                emit_kt(a, b)
                            elif kind == "qt":
                                emit_qT(a)
                            else:
                                emit_v(a)
                        for t in range(2):
                            h = 2 * p + t
                            nc.tensor.matmul(
                                ctx_pair[:, SH * t:SH * t + SH],
                                v_sb[j][:, (HD + 1) * h:(HD + 1) * h + HD + 1],
                                e[:, SH * t:SH * t + SH],
                                start=(j == 0), stop=(j == NJ - 1),
                            )
                        if j == 1 and pending_div is not None:
                            div_stage2(*pending_div, scp, "sc")
                            pending_div = None
                    zbf = div_stage1(p, ctx_pair)
                    if p < 3:
                        pending_div = (p, ctx_pair, zbf)
                    else:
                        last_div = (p, ctx_pair, zbf)

              # ---- out_proj + residual + LayerNorm per query tile ----
              # (scp is closed here; cxp stays open so pass 3's division can
              # finish while the p=0..2 out_proj partials run on freed scp
              # banks.)  The residual rides the PSUM accumulation (identity
              # matmul on x_nat); the final (y - mu) * rstd runs on the
              # scalar engine (Identity activation with per-row scale/bias).
              with tc.tile_pool(name="ops", bufs=1, space="PSUM") as ops:
                po_t = [ops.tile([128, D], F32, tag=f"po{qt}", name=f"po{qt}") for qt in range(4)]
                for qt in range(4):
                    nc.tensor.matmul(
                        po_t[qt], ident_sb, x_nat_sb[qt],
                        start=True, stop=False,
                    )
                for pp_i in range(3):
                    for qt in range(4):
                        nc.tensor.matmul(
                            po_t[qt],
                            ctxTs_sb[pp_i][:, 128 * qt:128 * qt + 128],
                            woutT_sb[pp_i],
                            start=False, stop=False,
                        )
                div_stage2(*last_div, cxp, "ctx")
                for qt in range(4):
                    po = po_t[qt]
                    nc.tensor.matmul(
                        po,
                        ctxTs_sb[3][:, 128 * qt:128 * qt + 128],
                        woutT_sb[3],
                        start=False, stop=True,
                    )
                    stats = wp.tile([128, 6], F32, tag="stats")
                    nc.vector.bn_stats(stats, po)
                    mv = wp.tile([128, 2], F32, tag="mv")
                    nc.vector.bn_aggr(mv, stats)
                    veps = wp.tile([128, 1], F32, tag="veps")
                    nc.vector.tensor_scalar_add(veps, mv[:, 1:2], eps_t)
                    rvar = wp.tile([128, 1], F32, tag="rvar")
                    nc.vector.reciprocal_approx_fast(rvar, veps)
                    rstd = wp.tile([128, 1], F32, tag="rstd")
                    nc.scalar.activation(rstd, rvar, AF.Sqrt)
                    nbias = wp.tile([128, 1], F32, tag="nbias")
                    nc.vector.tensor_scalar(
                        out=nbias, in0=mv[:, 0:1], scalar1=rstd, scalar2=-1.0,
                        op0=OP.mult, op1=OP.mult,
                    )
                    t1 = wp.tile([128, D], F32, tag="t1")
                    nc.scalar.activation(t1, po, AF.Identity, bias=nbias, scale=rstd)
                    if not ln_trivial:
                        nc.vector.tensor_tensor(out=t1, in0=t1, in1=gamma_sb, op=OP.mult)
                        nc.vector.tensor_tensor(out=t1, in0=t1, in1=beta_sb, op=OP.add)
                    nc.sync.dma_start(out=out_d[128 * qt:128 * qt + 128, :], in_=t1)

    nc.compile()
    return nc


def _host_prep(x, in_proj_w, in_proj_b, out_proj_w, out_proj_b, ln_gamma, ln_beta, window_size):
    x = np.ascontiguousarray(np.asarray(x, dtype=np.float32))
    in_proj_w = np.asarray(in_proj_w, dtype=np.float32)
    in_proj_b = np.asarray(in_proj_b, dtype=np.float32)
    out_proj_w = np.asarray(out_proj_w, dtype=np.float32)
    out_proj_b = np.asarray(out_proj_b, dtype=np.float32)
    ln_gamma = np.asarray(ln_gamma, dtype=np.float32)
    ln_beta = np.asarray(ln_beta, dtype=np.float32)
    w = int(np.asarray(window_size))
    half = w // 2
    assert half <= 128, "mask slots only cover |k-q| <= 128"

    bf16 = ml_dtypes.bfloat16

    def dpack(a):  # [4*128, C] -> [128, 4*C] partition-major d-chunk packing
        r, cc = a.shape
        return np.ascontiguousarray(
            a.reshape(4, 128, cc).transpose(1, 0, 2).reshape(128, 4 * cc)
        )

    scale = np.float32(1.0 / np.sqrt(HD))
    W = in_proj_w.copy()
    W[0:D] *= scale
    winT = dpack(np.ascontiguousarray(W.T)).astype(bf16)            # [128, 4*3D]
    woutT = dpack(np.ascontiguousarray(out_proj_w.T)).astype(bf16)  # [128, 4*D]
    bq = np.ascontiguousarray((in_proj_b[0:D] * scale).reshape(4, 128).T)  # [128, 4]
    bout = (out_proj_b + out_proj_w @ in_proj_b[2 * D:3 * D]).reshape(1, D)
    gamma_b = np.ascontiguousarray(np.broadcast_to(ln_gamma, (128, D)))
    beta_b = np.ascontiguousarray(np.broadcast_to(ln_beta, (128, D)))
    strips = []
    for j in MASK_SLOTS[:-1]:
        c0 = max(0, 128 * j - half + 1)
        c1 = min(SH, 128 * j + 128 + half)
        strips.append((c0, max(c1, c0 + 1)))
    strips.append((0, max(1, min(SH, half))))

    in_maps = []
    for c in range(8):
        b, s = divmod(c, 4)
        rot = (SH * s + np.arange(L)) % L
        xT_rot = dpack(np.ascontiguousarray(x[b][rot].T)).astype(bf16)  # [128, 4*L]
        x_nat = dpack(np.ascontiguousarray(
            x[b][SH * s:SH * s + SH] + bout[None, 0, :]
        )).astype(bf16)  # [128, 4*D]
        q_true = SH * s + np.arange(SH)[None, :]
        mask_cols = []
        for i, j in enumerate(MASK_SLOTS):
            k_true = (SH * s + 128 * j + np.arange(128)[:, None]) % L
            dd = k_true - q_true
            banned = (dd >= -half) & (dd < half)
            c0, c1 = strips[i]
            mask_cols.append((1.0 - banned[:, c0:c1].astype(np.float32)))
        maskp = np.concatenate(mask_cols, axis=1)
        m = {
            "xTp": xT_rot, "xnatp": x_nat, "winTp": winT, "woutTp": woutT,
            "bq": bq, "maskp": maskp.astype(bf16),
            "ident": np.eye(128, dtype=np.float32).astype(bf16),
        }
        if not LN_TRIVIAL:
            m["gamma"] = gamma_b
            m["beta"] = beta_b
        in_maps.append(m)
    return in_maps


def kernel(x, in_proj_w, in_proj_b, out_proj_w, out_proj_b, ln_gamma, ln_beta, window_size):
    global _COMPILED, LAST_RESULT
    half = int(np.asarray(window_size)) // 2
    ln_trivial = bool(np.all(np.asarray(ln_gamma) == 1.0) and np.all(np.asarray(ln_beta) == 0.0))
    key = (half, ln_trivial)
    if _COMPILED is None or _COMPILED[0] != key:
        _COMPILED = (key, _build(half, ln_trivial))
    in_maps = _host_prep(x, in_proj_w, in_proj_b, out_proj_w, out_proj_b,
                         ln_gamma, ln_beta, window_size)
    res = run_bass_kernel_spmd(_COMPILED[1], in_maps, core_ids=list(range(8)))
    LAST_RESULT = res
    out = np.empty((B, L, D), np.float32)
    for c in range(8):
        b, s = divmod(c, 4)
        out[b, SH * s:SH * s + SH] = res.results[c]["out"]
    return out


# revision 43
# speedup vs baseline: 1.0307x; 1.0118x over previous
"""Trainium2 Bass kernel for LocalSparseAttention (anti-local windowed attention).

Reference computation (B=2, L=2048, D=512, H=8, hd=64):
    qkv = x @ in_proj_w.T + in_proj_b ; q,k,v = split(qkv)
    q *= 1/sqrt(hd)
    scores = q @ k.T  per head, with positions j in [i-w/2, i+w/2) BANNED (-inf)
    attn = softmax(scores); ctx = attn @ v
    out = LayerNorm(x + ctx @ out_proj_w.T + out_proj_b) * gamma + beta

Sharding: 8 cores = 2 batches x 4 query-shards of 512 rows. Each core
computes k/v for all 2048 keys of its batch (from a host-rotated x^T so
the banned diagonal band lands at fixed key-tile loop positions on every
core, keeping the SPMD graph uniform; masks are per-core 0/1 input data),
and full attention + out_proj + residual + LayerNorm for its 512 queries.

Math transformations (validated vs the reference):
  - k-bias dropped: softmax invariant.
  - v-bias folded into out_proj bias (attn rows sum to 1).
  - q scaled by 1/sqrt(hd) by scaling Wq/bq on host.
  - no max-subtraction in softmax (scores ~ N(0,1), exp safe); banned
    positions zeroed AFTER exp via 0/1 mask multiply.
  - softmax denominator via a ones-column appended to v (row 64 of the
    65-row ctx accumulator); divided out with a PE outer-product
    broadcast + DVE fast-reciprocal.

Datapath is bf16 (weights, x^T, k^T, v, q^T, exp(scores), masks, ctx)
with fp32 PSUM accumulation; residual x and LayerNorm stay fp32.  bf16
doubles PE streaming + LDWEIGHTS rate (FWL) vs fp32 and halves DMA.

Structure: 4 passes of (2 heads x 16 key tiles), each pass owning one
128-row chunk of q^T/k^T.  The softmax division of pass p overlaps pass
p+1's score/ctx matmuls (ping-pong PSUM ctx slots); k^T/v prep matmuls
are deadline-interleaved into the pass loops.
"""

import ml_dtypes
import numpy as np

import concourse.bass as bass
import concourse.tile as tile
import concourse.mybir as mybir
from concourse import bacc
from concourse.bass_utils import run_bass_kernel_spmd

F32 = mybir.dt.float32
BF16 = mybir.dt.bfloat16
AF = mybir.ActivationFunctionType
OP = mybir.AluOpType

B, L, D = 2, 2048, 512
H, HD = 8, 64
SH = L // 4            # 512-query shard per core
NJ = 16                # key tiles of 128 per sequence
MASK_SLOTS = [0, 1, 2, 3, 4, 15]   # key-tile loop positions that can carry the band
LN_EPS = 1e-5

USE_POOL_MASKS = False  # GpSimd ops need ucode libraries this stack lacks; keep DVE

_COMPILED = None
LAST_RESULT = None
STRIPS = []
LN_TRIVIAL = False


def _build(half, ln_trivial):
    global LN_TRIVIAL, STRIPS
    LN_TRIVIAL = ln_trivial
    STRIPS = []
    for j in MASK_SLOTS[:-1]:
        c0 = max(0, 128 * j - half + 1)
        c1 = min(SH, 128 * j + 128 + half)
        STRIPS.append((c0, max(c1, c0 + 1)))
    STRIPS.append((0, max(1, min(SH, half))))

    nc = bacc.Bacc("TRN2", target_bir_lowering=False, debug=False, num_devices=8)

    # All inputs are host-packed partition-major ([128, ...]) so each
    # logical load is ONE dma_start — the Sync queue serializes dma_start
    # instructions at ~0.6us each, so instruction count is what matters.
    strip_w = [c1 - c0 for c0, c1 in STRIPS]
    W_MASK = sum(strip_w)
    xT = nc.dram_tensor("xTp", [128, 4 * L], BF16, kind="ExternalInput")       # rotated x^T, d-chunk packed
    x_nat = nc.dram_tensor("xnatp", [128, 4 * D], BF16, kind="ExternalInput")  # query rows + folded out bias
    ident_d = nc.dram_tensor("ident", [128, 128], BF16, kind="ExternalInput")
    winT = nc.dram_tensor("winTp", [128, 4 * 3 * D], BF16, kind="ExternalInput")  # in_proj_w.T, q pre-scaled
    woutT = nc.dram_tensor("woutTp", [128, 4 * D], BF16, kind="ExternalInput")    # out_proj_w.T
    bq_d = nc.dram_tensor("bq", [128, 4], F32, kind="ExternalInput")           # scaled q bias, chunked
    masks_d = nc.dram_tensor("maskp", [128, W_MASK], BF16, kind="ExternalInput")
    if not ln_trivial:
        gamma_d = nc.dram_tensor("gamma", [128, D], F32, kind="ExternalInput")
        beta_d = nc.dram_tensor("beta", [128, D], F32, kind="ExternalInput")
    out_d = nc.dram_tensor("out", [SH, D], F32, kind="ExternalOutput")

    mask_engine = None  # resolved inside

    with tile.TileContext(nc) as tc:
        with (
            tc.tile_pool(name="persist", bufs=1) as pp,
            tc.tile_pool(name="work", bufs=2) as wp,
            tc.tile_pool(name="expp", bufs=4) as ep,
        ):
            mask_engine = nc.gpsimd if USE_POOL_MASKS else nc.vector

            # ---- DMA: one instruction per logical load; critical-path
            # bytes (x^T seg 0, q/k weight chunk 0, v weights) first ----
            bq_sb = pp.tile([128, 4], F32, tag="bq")
            nc.sync.dma_start(out=bq_sb, in_=bq_d[:, :])
            xT_all = pp.tile([128, 4 * L], BF16, tag="xT", name="xT_all")
            wq_all = pp.tile([128, 4 * D], BF16, tag="wq", name="wq_all")
            kv_all = pp.tile([128, 4 * 2 * D], BF16, tag="kv", name="kv_all")
            xT_r = xT_all.rearrange("p (d c) -> p d c", d=4)
            xTd_r = xT.rearrange("p (d c) -> p d c", d=4)
            wq_r = wq_all.rearrange("p (d c) -> p d c", d=4)
            kv_r = kv_all.rearrange("p (d c) -> p d c", d=4)
            win_r = winT.rearrange("p (d c) -> p d c", d=4)
            # x^T seg 0 (q^T + kt0 seg0 + first v preps)
            nc.sync.dma_start(out=xT_r[:, :, 0:512], in_=xTd_r[:, :, 0:512])
            # q-weight chunk 0, k-weight chunk 0, v weights
            nc.sync.dma_start(out=wq_r[:, :, 0:128], in_=win_r[:, :, 0:128])
            nc.sync.dma_start(out=kv_r[:, :, 0:128], in_=win_r[:, :, 512:640])
            nc.sync.dma_start(out=kv_r[:, :, 512:1024], in_=win_r[:, :, 1024:1536])
            # masks (packed strips)
            mask_all = pp.tile([128, W_MASK], BF16, tag="maskp", name="mask_all")
            nc.sync.dma_start(out=mask_all, in_=masks_d[:, :])
            mask_sb = []
            moff = 0
            for i in range(len(MASK_SLOTS)):
                mask_sb.append(mask_all[:, moff:moff + strip_w[i]])
                moff += strip_w[i]
            # x^T segs 1-3, remaining q/k weight chunks
            nc.sync.dma_start(out=xT_r[:, :, 512:2048], in_=xTd_r[:, :, 512:2048])
            nc.sync.dma_start(out=wq_r[:, :, 128:512], in_=win_r[:, :, 128:512])
            nc.sync.dma_start(out=kv_r[:, :, 128:512], in_=win_r[:, :, 640:1024])
            wout_all = pp.tile([128, 4 * D], BF16, tag="woutp", name="wout_all")
            nc.sync.dma_start(out=wout_all, in_=woutT[:, :])
            woutT_sb = [wout_all[:, 512 * p:512 * p + 512] for p in range(4)]
            xnat_all = pp.tile([128, 4 * D], BF16, tag="xnatp", name="xnat_all")
            nc.sync.dma_start(out=xnat_all, in_=x_nat[:, :])
            x_nat_sb = [xnat_all[:, 512 * qt:512 * qt + 512] for qt in range(4)]
            ident_sb = pp.tile([128, 128], BF16, tag="ident")
            nc.sync.dma_start(out=ident_sb, in_=ident_d[:, :])
            if not ln_trivial:
                gamma_sb = pp.tile([128, D], F32, tag="gamma")
                nc.sync.dma_start(out=gamma_sb, in_=gamma_d[:, :])
                beta_sb = pp.tile([128, D], F32, tag="beta")
                nc.sync.dma_start(out=beta_sb, in_=beta_d[:, :])

            # ---- constants ----
            wupf = pp.tile([128, 256], F32, tag="wupf")
            nc.vector.memset(wupf, 0.001)
            wup = pp.tile([128, 256], BF16, tag="wup")
            nc.vector.tensor_copy(wup, wupf)
            ones_bf = pp.tile([1, 64], BF16, tag="ones1")
            nc.vector.memset(ones_bf, 1.0)
            ones8 = pp.tile([128, 8], BF16, tag="ones8")
            nc.vector.memset(ones8, 1.0)
            eps_t = pp.tile([128, 1], F32, tag="eps")
            nc.vector.memset(eps_t, LN_EPS)

            ctxTs_sb = [pp.tile([128, SH], BF16, tag=f"ctxTs{p}", name=f"ctxTs{p}") for p in range(4)]
            kt_sb = [pp.tile([128, L], BF16, tag=f"kt{c}", name=f"kt{c}") for c in range(4)]
            v_sb = [pp.tile([128, H * (HD + 1)], BF16, tag=f"v{l2}", name=f"v{l2}") for l2 in range(NJ)]

            # PE warm-up: keep the PE busy (and the HAM clock-gate window
            # filling) while the first input DMAs land.
            with tc.tile_pool(name="wups", bufs=1, space="PSUM") as wps:
                wu_ps = wps.tile([128, 256], F32, tag="wu")
                for i in range(12):
                    nc.tensor.matmul(
                        wu_ps, wup[:, 0:128], wup,
                        start=(i == 0), stop=(i == 11),
                    )

            with tc.tile_pool(name="cxp", bufs=2, space="PSUM") as cxp:
              with tc.tile_pool(name="scp", bufs=2, space="PSUM") as scp:
                # ---- q^T: [D, SH] as 4 chunks of [128, SH]; chunk c is
                # only needed by pass c, so chunks 1-3 are emitted late ----
                qT_sb = [None] * 4

                def emit_qT(c):
                    ps = scp.tile([128, SH], F32, tag="sc", name=f"qps{c}")
                    for d in range(4):
                        nc.tensor.matmul(
                            ps,
                            wq_all[:, 512 * d + 128 * c:512 * d + 128 * c + 128],
                            xT_all[:, 2048 * d:2048 * d + SH],
                            start=(d == 0), stop=(d == 3),
                        )
                    qt = pp.tile([128, SH], BF16, tag=f"qT{c}", name=f"qT{c}")
                    nc.vector.tensor_scalar_add(qt, ps, bq_sb[:, c:c + 1])
                    qT_sb[c] = qt

                def emit_kt(c2, seg):
                    ps = scp.tile([128, 512], F32, tag="sc", name=f"ktps{c2}_{seg}")
                    for d in range(4):
                        nc.tensor.matmul(
                            ps,
                            kv_all[:, 1024 * d + 128 * c2:1024 * d + 128 * c2 + 128],
                            xT_all[:, 2048 * d + 512 * seg:2048 * d + 512 * seg + 512],
                            start=(d == 0), stop=(d == 3),
                        )
                    nc.vector.tensor_copy(kt_sb[c2][:, 512 * seg:512 * seg + 512], ps)

                def emit_v(l2):
                    ps = scp.tile([128, 512], F32, tag="sc", name=f"vps{l2}")
                    for d in range(4):
                        nc.tensor.matmul(
                            ps,
                            xT_all[:, 2048 * d + 128 * l2:2048 * d + 128 * l2 + 128],
                            kv_all[:, 1024 * d + 512:1024 * d + 1024],
                            start=(d == 0), stop=(d == 3),
                        )
                    vr = v_sb[l2].rearrange("p (t c) -> p t c", c=HD + 1)
                    nc.vector.tensor_copy(
                        vr[:, :, HD:HD + 1],
                        ones8.rearrange("p (t c) -> p t c", c=1),
                    )
                    nc.vector.tensor_copy(
                        vr[:, :, 0:HD],
                        ps.rearrange("p (t c) -> p t c", c=HD),
                    )

                # prep deadlines: v[l2] first used at pass 0 iter l2; kt
                # chunk c seg s first used at pass c iter 4s; qT chunk c at
                # pass c iter 0.  Keep the pre-pass minimal so the first
                # exp fires early, and level the rest into the pass loops.
                emit_qT(0)
                emit_kt(0, 0)
                prep_at = {}
                for l2 in range(NJ):
                    prep_at.setdefault((0, max(0, l2 - 2)), []).append(("v", l2, None))
                for s in range(1, 4):
                    prep_at.setdefault((0, 4 * s - 3), []).append(("kt", 0, s))
                for c in range(1, 4):
                    prep_at.setdefault((c - 1, 12), []).append(("qt", c, None))
                    prep_at.setdefault((c - 1, 13), []).append(("kt", c, 0))
                    for s in range(1, 4):
                        prep_at.setdefault((c, 4 * s - 3), []).append(("kt", c, s))

                # softmax division: Z-row cast to SBUF (DVE), PE outer-product
                # broadcast, fast-reciprocal + multiply (DVE).  The PE ops of
                # pass p's division are deferred until pass p+1's iter 1 so
                # the in-order PE queue never stalls on the Z cast.
                def div_stage1(p, ctx_pair):
                    zbf = wp.tile([1, 2 * SH], BF16, tag="zbf", name=f"zbf{p}")
                    nc.vector.tensor_copy(zbf, ctx_pair[HD:HD + 1, :])
                    return zbf

                def div_stage2(p, ctx_pair, zbf, pool, pool_tag):
                    bc = pool.tile([64, 2 * SH], F32, tag=pool_tag, name=f"bc{p}")
                    for t in range(2):
                        nc.tensor.matmul(
                            bc[:, 512 * t:512 * t + 512],
                            ones_bf[0:1, 0:HD],
                            zbf[0:1, 512 * t:512 * t + 512],
                            start=True, stop=True,
                        )
                    rb = wp.tile([64, 2 * SH], F32, tag="rb", name=f"rb{p}")
                    nc.vector.reciprocal_approx_fast(rb, bc)
                    for t in range(2):
                        nc.vector.tensor_tensor(
                            out=ctxTs_sb[p][64 * t:64 * t + 64, :],
                            in0=ctx_pair[0:HD, 512 * t:512 * t + 512],
                            in1=rb[:, 512 * t:512 * t + 512],
                            op=OP.mult,
                        )

                pending_div = None
                # Keep the sc-slot A/B alternation intact: prep/bc tiles
                # share the "sc" tag rotation, and an ODD number of them
                # between consecutive score tiles makes the next scores
                # land on the just-read slot (a ~2us WAR stall on exp).
                # A zero-cost dummy allocation restores parity.
                alloc_ct = [0]

                for p in range(4):
                    ctx_pair = cxp.tile([HD + 1, 2 * SH], F32, tag="ctx", name=f"ctx{p}")
                    for j in range(NJ):
                        sc = scp.tile([128, 2 * SH], F32, tag="sc")
                        for t in range(2):
                            nc.tensor.matmul(
                                sc[:, SH * t:SH * t + SH],
                                kt_sb[p][64 * t:64 * t + 64, 128 * j:128 * j + 128],
                                qT_sb[p][64 * t:64 * t + 64, :],
                                start=True, stop=True,
                            )
                        e = ep.tile([128, 2 * SH], BF16, tag="exp")
                        nc.scalar.activation(e, sc, AF.Exp)
                        if j in MASK_SLOTS:
                            slot = MASK_SLOTS.index(j)
                            c0, c1 = STRIPS[slot]
                            for t in range(2):
                                mask_engine.tensor_tensor(
                                    out=e[:, SH * t + c0:SH * t + c1],
                                    in0=e[:, SH * t + c0:SH * t + c1],
                                    in1=mask_sb[slot],
                                    op=OP.mult,
                                )
                        # prep matmuls AFTER this iter's scores: they fill
                        # the PE while exp runs instead of delaying it
                        for kind, a, b in prep_at.get((p, j), []):
                            if kind == "kt":
                                emit_kt(a, b)
                            elif kind == "qt":
                                emit_qT(a)
                            else:
                                emit_v(a)
                            alloc_ct[0] += 1
                        for t in range(2):
                            h = 2 * p + t
                            nc.tensor.matmul(
                                ctx_pair[:, SH * t:SH * t + SH],
                                v_sb[j][:, (HD + 1) * h:(HD + 1) * h + HD + 1],
                                e[:, SH * t:SH * t + SH],
                                start=(j == 0), stop=(j == NJ - 1),
                            )
                        if j == 1 and pending_div is not None:
                            div_stage2(*pending_div, scp, "sc")
                            alloc_ct[0] += 1
                            pending_div = None
                        if alloc_ct[0] % 2 == 1:
                            scp.tile([128, 8], F32, tag="sc", name=f"par{p}_{j}")
                        alloc_ct[0] = 0
                    zbf = div_stage1(p, ctx_pair)
                    if p < 3:
                        pending_div = (p, ctx_pair, zbf)
                    else:
                        last_div = (p, ctx_pair, zbf)

              # ---- out_proj + residual + LayerNorm per query tile ----
              # (scp is closed here; cxp stays open so pass 3's division can
              # finish while the p=0..2 out_proj partials run on freed scp
              # banks.)  The residual rides the PSUM accumulation (identity
              # matmul on x_nat); the final (y - mu) * rstd runs on the
              # scalar engine (Identity activation with per-row scale/bias).
              with tc.tile_pool(name="ops", bufs=1, space="PSUM") as ops:
                po_t = [ops.tile([128, D], F32, tag=f"po{qt}", name=f"po{qt}") for qt in range(4)]
                for qt in range(4):
                    nc.tensor.matmul(
                        po_t[qt], ident_sb, x_nat_sb[qt],
                        start=True, stop=False,
                    )
                for pp_i in range(3):
                    for qt in range(4):
                        nc.tensor.matmul(
                            po_t[qt],
                            ctxTs_sb[pp_i][:, 128 * qt:128 * qt + 128],
                            woutT_sb[pp_i],
                            start=False, stop=False,
                        )
                div_stage2(*last_div, cxp, "ctx")
                for qt in range(4):
                    po = po_t[qt]
                    nc.tensor.matmul(
                        po,
                        ctxTs_sb[3][:, 128 * qt:128 * qt + 128],
                        woutT_sb[3],
                        start=False, stop=True,
                    )
                    stats = wp.tile([128, 6], F32, tag="stats")
                    nc.vector.bn_stats(stats, po)
                    mv = wp.tile([128, 2], F32, tag="mv")
                    nc.vector.bn_aggr(mv, stats)
                    veps = wp.tile([128, 1], F32, tag="veps")
                    nc.vector.tensor_scalar_add(veps, mv[:, 1:2], eps_t)
                    rvar = wp.tile([128, 1], F32, tag="rvar")
                    nc.vector.reciprocal_approx_fast(rvar, veps)
                    rstd = wp.tile([128, 1], F32, tag="rstd")
                    nc.scalar.activation(rstd, rvar, AF.Sqrt)
                    nbias = wp.tile([128, 1], F32, tag="nbias")
                    nc.vector.tensor_scalar(
                        out=nbias, in0=mv[:, 0:1], scalar1=rstd, scalar2=-1.0,
                        op0=OP.mult, op1=OP.mult,
                    )
                    t1 = wp.tile([128, D], F32, tag="t1")
                    nc.scalar.activation(t1, po, AF.Identity, bias=nbias, scale=rstd)
                    if not ln_trivial:
                        nc.vector.tensor_tensor(out=t1, in0=t1, in1=gamma_sb, op=OP.mult)
                        nc.vector.tensor_tensor(out=t1, in0=t1, in1=beta_sb, op=OP.add)
                    nc.sync.dma_start(out=out_d[128 * qt:128 * qt + 128, :], in_=t1)

    nc.compile()
    return nc


def _host_prep(x, in_proj_w, in_proj_b, out_proj_w, out_proj_b, ln_gamma, ln_beta, window_size):
    x = np.ascontiguousarray(np.asarray(x, dtype=np.float32))
    in_proj_w = np.asarray(in_proj_w, dtype=np.float32)
    in_proj_b = np.asarray(in_proj_b, dtype=np.float32)
    out_proj_w = np.asarray(out_proj_w, dtype=np.float32)
    out_proj_b = np.asarray(out_proj_b, dtype=np.float32)
    ln_gamma = np.asarray(ln_gamma, dtype=np.float32)
    ln_beta = np.asarray(ln_beta, dtype=np.float32)
    w = int(np.asarray(window_size))
    half = w // 2
    assert half <= 128, "mask slots only cover |k-q| <= 128"

    bf16 = ml_dtypes.bfloat16

    def dpack(a):  # [4*128, C] -> [128, 4*C] partition-major d-chunk packing
        r, cc = a.shape
        return np.ascontiguousarray(
            a.reshape(4, 128, cc).transpose(1, 0, 2).reshape(128, 4 * cc)
        )

    scale = np.float32(1.0 / np.sqrt(HD))
    W = in_proj_w.copy()
    W[0:D] *= scale
    winT = dpack(np.ascontiguousarray(W.T)).astype(bf16)            # [128, 4*3D]
    woutT = dpack(np.ascontiguousarray(out_proj_w.T)).astype(bf16)  # [128, 4*D]
    bq = np.ascontiguousarray((in_proj_b[0:D] * scale).reshape(4, 128).T)  # [128, 4]
    bout = (out_proj_b + out_proj_w @ in_proj_b[2 * D:3 * D]).reshape(1, D)
    gamma_b = np.ascontiguousarray(np.broadcast_to(ln_gamma, (128, D)))
    beta_b = np.ascontiguousarray(np.broadcast_to(ln_beta, (128, D)))
    strips = []
    for j in MASK_SLOTS[:-1]:
        c0 = max(0, 128 * j - half + 1)
        c1 = min(SH, 128 * j + 128 + half)
        strips.append((c0, max(c1, c0 + 1)))
    strips.append((0, max(1, min(SH, half))))

    in_maps = []
    for c in range(8):
        b, s = divmod(c, 4)
        rot = (SH * s + np.arange(L)) % L
        xT_rot = dpack(np.ascontiguousarray(x[b][rot].T)).astype(bf16)  # [128, 4*L]
        x_nat = dpack(np.ascontiguousarray(
            x[b][SH * s:SH * s + SH] + bout[None, 0, :]
        )).astype(bf16)  # [128, 4*D]
        q_true = SH * s + np.arange(SH)[None, :]
        mask_cols = []
        for i, j in enumerate(MASK_SLOTS):
            k_true = (SH * s + 128 * j + np.arange(128)[:, None]) % L
            dd = k_true - q_true
            banned = (dd >= -half) & (dd < half)
            c0, c1 = strips[i]
            mask_cols.append((1.0 - banned[:, c0:c1].astype(np.float32)))
        maskp = np.concatenate(mask_cols, axis=1)
        m = {
            "xTp": xT_rot, "xnatp": x_nat, "winTp": winT, "woutTp": woutT,
            "bq": bq, "maskp": maskp.astype(bf16),
            "ident": np.eye(128, dtype=np.float32).astype(bf16),
        }
        if not LN_TRIVIAL:
            m["gamma"] = gamma_b
            m["beta"] = beta_b
        in_maps.append(m)
    return in_maps


def kernel(x, in_proj_w, in_proj_b, out_proj_w, out_proj_b, ln_gamma, ln_beta, window_size):
    global _COMPILED, LAST_RESULT
    half = int(np.asarray(window_size)) // 2
    ln_trivial = bool(np.all(np.asarray(ln_gamma) == 1.0) and np.all(np.asarray(ln_beta) == 0.0))
    key = (half, ln_trivial)
    if _COMPILED is None or _COMPILED[0] != key:
        _COMPILED = (key, _build(half, ln_trivial))
    in_maps = _host_prep(x, in_proj_w, in_proj_b, out_proj_w, out_proj_b,
                         ln_gamma, ln_beta, window_size)
    res = run_bass_kernel_spmd(_COMPILED[1], in_maps, core_ids=list(range(8)))
    LAST_RESULT = res
    out = np.empty((B, L, D), np.float32)
    for c in range(8):
        b, s = divmod(c, 4)
        out[b, SH * s:SH * s + SH] = res.results[c]["out"]
    return out


# revision 44
# speedup vs baseline: 1.0460x; 1.0148x over previous
"""Trainium2 Bass kernel for LocalSparseAttention (anti-local windowed attention).

Reference computation (B=2, L=2048, D=512, H=8, hd=64):
    qkv = x @ in_proj_w.T + in_proj_b ; q,k,v = split(qkv)
    q *= 1/sqrt(hd)
    scores = q @ k.T  per head, with positions j in [i-w/2, i+w/2) BANNED (-inf)
    attn = softmax(scores); ctx = attn @ v
    out = LayerNorm(x + ctx @ out_proj_w.T + out_proj_b) * gamma + beta

Sharding: 8 cores = 2 batches x 4 query-shards of 512 rows. Each core
computes k/v for all 2048 keys of its batch (from a host-rotated x^T so
the banned diagonal band lands at fixed key-tile loop positions on every
core, keeping the SPMD graph uniform; masks are per-core 0/1 input data),
and full attention + out_proj + residual + LayerNorm for its 512 queries.

Math transformations (validated vs the reference):
  - k-bias dropped: softmax invariant.
  - v-bias folded into out_proj bias (attn rows sum to 1).
  - q scaled by 1/sqrt(hd) by scaling Wq/bq on host.
  - no max-subtraction in softmax (scores ~ N(0,1), exp safe); banned
    positions zeroed AFTER exp via 0/1 mask multiply.
  - softmax denominator via a ones-column appended to v (row 64 of the
    65-row ctx accumulator); divided out with a PE outer-product
    broadcast + DVE fast-reciprocal.

Datapath is bf16 (weights, x^T, k^T, v, q^T, exp(scores), masks, ctx)
with fp32 PSUM accumulation; residual x and LayerNorm stay fp32.  bf16
doubles PE streaming + LDWEIGHTS rate (FWL) vs fp32 and halves DMA.

Structure: 4 passes of (2 heads x 16 key tiles), each pass owning one
128-row chunk of q^T/k^T.  The softmax division of pass p overlaps pass
p+1's score/ctx matmuls (ping-pong PSUM ctx slots); k^T/v prep matmuls
are deadline-interleaved into the pass loops.
"""

import ml_dtypes
import numpy as np

import concourse.bass as bass
import concourse.tile as tile
import concourse.mybir as mybir
from concourse import bacc
from concourse.bass_utils import run_bass_kernel_spmd

F32 = mybir.dt.float32
BF16 = mybir.dt.bfloat16
AF = mybir.ActivationFunctionType
OP = mybir.AluOpType

B, L, D = 2, 2048, 512
H, HD = 8, 64
SH = L // 4            # 512-query shard per core
NJ = 16                # key tiles of 128 per sequence
MASK_SLOTS = [0, 1, 2, 3, 4, 15]   # key-tile loop positions that can carry the band
LN_EPS = 1e-5

USE_POOL_MASKS = False  # GpSimd ops need ucode libraries this stack lacks; keep DVE

_COMPILED = None
LAST_RESULT = None
STRIPS = []
LN_TRIVIAL = False


def _build(half, ln_trivial):
    global LN_TRIVIAL, STRIPS
    LN_TRIVIAL = ln_trivial
    STRIPS = []
    for j in MASK_SLOTS[:-1]:
        c0 = max(0, 128 * j - half + 1)
        c1 = min(SH, 128 * j + 128 + half)
        STRIPS.append((c0, max(c1, c0 + 1)))
    STRIPS.append((0, max(1, min(SH, half))))

    nc = bacc.Bacc("TRN2", target_bir_lowering=False, debug=False, num_devices=8)

    # All inputs are host-packed partition-major ([128, ...]) so each
    # logical load is ONE dma_start — the Sync queue serializes dma_start
    # instructions at ~0.6us each, so instruction count is what matters.
    strip_w = [c1 - c0 for c0, c1 in STRIPS]
    W_MASK = sum(strip_w)
    xT = nc.dram_tensor("xTp", [128, 4 * L], BF16, kind="ExternalInput")       # rotated x^T, d-chunk packed
    x_nat = nc.dram_tensor("xnatp", [128, 4 * D], BF16, kind="ExternalInput")  # query rows + folded out bias
    ident_d = nc.dram_tensor("ident", [128, 128], BF16, kind="ExternalInput")
    winT = nc.dram_tensor("winTp", [128, 4 * 3 * D], BF16, kind="ExternalInput")  # in_proj_w.T, q pre-scaled
    woutT = nc.dram_tensor("woutTp", [128, 4 * D], BF16, kind="ExternalInput")    # out_proj_w.T
    bq_d = nc.dram_tensor("bq", [128, 4], F32, kind="ExternalInput")           # scaled q bias, chunked
    masks_d = nc.dram_tensor("maskp", [128, W_MASK], BF16, kind="ExternalInput")
    if not ln_trivial:
        gamma_d = nc.dram_tensor("gamma", [128, D], F32, kind="ExternalInput")
        beta_d = nc.dram_tensor("beta", [128, D], F32, kind="ExternalInput")
    out_d = nc.dram_tensor("out", [SH, D], F32, kind="ExternalOutput")

    mask_engine = None  # resolved inside

    with tile.TileContext(nc) as tc:
        with (
            tc.tile_pool(name="persist", bufs=1) as pp,
            tc.tile_pool(name="work", bufs=2) as wp,
            tc.tile_pool(name="expp", bufs=4) as ep,
        ):
            mask_engine = nc.gpsimd if USE_POOL_MASKS else nc.vector

            # ---- DMA: one instruction per logical load; critical-path
            # bytes (x^T seg 0, q/k weight chunk 0, v weights) first ----
            bq_sb = pp.tile([128, 4], F32, tag="bq")
            nc.sync.dma_start(out=bq_sb, in_=bq_d[:, :])
            xT_all = pp.tile([128, 4 * L], BF16, tag="xT", name="xT_all")
            wq_all = pp.tile([128, 4 * D], BF16, tag="wq", name="wq_all")
            kv_all = pp.tile([128, 4 * 2 * D], BF16, tag="kv", name="kv_all")
            xT_r = xT_all.rearrange("p (d c) -> p d c", d=4)
            xTd_r = xT.rearrange("p (d c) -> p d c", d=4)
            wq_r = wq_all.rearrange("p (d c) -> p d c", d=4)
            kv_r = kv_all.rearrange("p (d c) -> p d c", d=4)
            win_r = winT.rearrange("p (d c) -> p d c", d=4)
            # x^T seg 0 (q^T + kt0 seg0 + first v preps)
            nc.sync.dma_start(out=xT_r[:, :, 0:512], in_=xTd_r[:, :, 0:512])
            # q-weight chunk 0, k-weight chunk 0, v weights
            nc.sync.dma_start(out=wq_r[:, :, 0:128], in_=win_r[:, :, 0:128])
            nc.sync.dma_start(out=kv_r[:, :, 0:128], in_=win_r[:, :, 512:640])
            nc.sync.dma_start(out=kv_r[:, :, 512:1024], in_=win_r[:, :, 1024:1536])
            # masks (packed strips)
            mask_all = pp.tile([128, W_MASK], BF16, tag="maskp", name="mask_all")
            nc.sync.dma_start(out=mask_all, in_=masks_d[:, :])
            mask_sb = []
            moff = 0
            for i in range(len(MASK_SLOTS)):
                mask_sb.append(mask_all[:, moff:moff + strip_w[i]])
                moff += strip_w[i]
            # x^T segs 1-3, remaining q/k weight chunks
            nc.sync.dma_start(out=xT_r[:, :, 512:2048], in_=xTd_r[:, :, 512:2048])
            nc.sync.dma_start(out=wq_r[:, :, 128:512], in_=win_r[:, :, 128:512])
            nc.sync.dma_start(out=kv_r[:, :, 128:512], in_=win_r[:, :, 640:1024])
            wout_all = pp.tile([128, 4 * D], BF16, tag="woutp", name="wout_all")
            nc.sync.dma_start(out=wout_all, in_=woutT[:, :])
            woutT_sb = [wout_all[:, 512 * p:512 * p + 512] for p in range(4)]
            xnat_all = pp.tile([128, 4 * D], BF16, tag="xnatp", name="xnat_all")
            nc.sync.dma_start(out=xnat_all, in_=x_nat[:, :])
            x_nat_sb = [xnat_all[:, 512 * qt:512 * qt + 512] for qt in range(4)]
            ident_sb = pp.tile([128, 128], BF16, tag="ident")
            nc.sync.dma_start(out=ident_sb, in_=ident_d[:, :])
            if not ln_trivial:
                gamma_sb = pp.tile([128, D], F32, tag="gamma")
                nc.sync.dma_start(out=gamma_sb, in_=gamma_d[:, :])
                beta_sb = pp.tile([128, D], F32, tag="beta")
                nc.sync.dma_start(out=beta_sb, in_=beta_d[:, :])

            # ---- constants ----
            wupf = pp.tile([128, 256], F32, tag="wupf")
            nc.vector.memset(wupf, 0.001)
            wup = pp.tile([128, 256], BF16, tag="wup")
            nc.vector.tensor_copy(wup, wupf)
            ones_bf = pp.tile([1, 64], BF16, tag="ones1")
            nc.vector.memset(ones_bf, 1.0)
            ones8 = pp.tile([128, 8], BF16, tag="ones8")
            nc.vector.memset(ones8, 1.0)
            eps_t = pp.tile([128, 1], F32, tag="eps")
            nc.vector.memset(eps_t, LN_EPS)

            ctxTs_sb = [pp.tile([128, SH], BF16, tag=f"ctxTs{p}", name=f"ctxTs{p}") for p in range(4)]
            kt_sb = [pp.tile([128, L], BF16, tag=f"kt{c}", name=f"kt{c}") for c in range(4)]
            v_sb = [pp.tile([128, H * (HD + 1)], BF16, tag=f"v{l2}", name=f"v{l2}") for l2 in range(NJ)]

            # PE warm-up: keep the PE busy (and the HAM clock-gate window
            # filling) while the first input DMAs land.
            with tc.tile_pool(name="wups", bufs=1, space="PSUM") as wps:
                # long enough to bridge until the first input DMAs land
                # (~12.8us) so the PE never idles and the HAM clock gate
                # opens before the real matmuls start
                wu_ps = wps.tile([128, 256], F32, tag="wu")
                for i in range(22):
                    nc.tensor.matmul(
                        wu_ps, wup[:, 0:128], wup,
                        start=(i == 0), stop=(i == 21),
                    )

            with tc.tile_pool(name="cxp", bufs=2, space="PSUM") as cxp:
              with tc.tile_pool(name="scp", bufs=2, space="PSUM") as scp:
                # ---- q^T: [D, SH] as 4 chunks of [128, SH]; chunk c is
                # only needed by pass c, so chunks 1-3 are emitted late ----
                qT_sb = [None] * 4

                def emit_qT(c):
                    ps = scp.tile([128, SH], F32, tag="sc", name=f"qps{c}")
                    for d in range(4):
                        nc.tensor.matmul(
                            ps,
                            wq_all[:, 512 * d + 128 * c:512 * d + 128 * c + 128],
                            xT_all[:, 2048 * d:2048 * d + SH],
                            start=(d == 0), stop=(d == 3),
                        )
                    qt = pp.tile([128, SH], BF16, tag=f"qT{c}", name=f"qT{c}")
                    nc.vector.tensor_scalar_add(qt, ps, bq_sb[:, c:c + 1])
                    qT_sb[c] = qt

                def emit_kt(c2, seg):
                    ps = scp.tile([128, 512], F32, tag="sc", name=f"ktps{c2}_{seg}")
                    for d in range(4):
                        nc.tensor.matmul(
                            ps,
                            kv_all[:, 1024 * d + 128 * c2:1024 * d + 128 * c2 + 128],
                            xT_all[:, 2048 * d + 512 * seg:2048 * d + 512 * seg + 512],
                            start=(d == 0), stop=(d == 3),
                        )
                    nc.vector.tensor_copy(kt_sb[c2][:, 512 * seg:512 * seg + 512], ps)

                def emit_v(l2):
                    ps = scp.tile([128, 512], F32, tag="sc", name=f"vps{l2}")
                    for d in range(4):
                        nc.tensor.matmul(
                            ps,
                            xT_all[:, 2048 * d + 128 * l2:2048 * d + 128 * l2 + 128],
                            kv_all[:, 1024 * d + 512:1024 * d + 1024],
                            start=(d == 0), stop=(d == 3),
                        )
                    vr = v_sb[l2].rearrange("p (t c) -> p t c", c=HD + 1)
                    nc.vector.tensor_copy(
                        vr[:, :, HD:HD + 1],
                        ones8.rearrange("p (t c) -> p t c", c=1),
                    )
                    nc.vector.tensor_copy(
                        vr[:, :, 0:HD],
                        ps.rearrange("p (t c) -> p t c", c=HD),
                    )

                # prep deadlines: v[l2] first used at pass 0 iter l2; kt
                # chunk c seg s first used at pass c iter 4s; qT chunk c at
                # pass c iter 0.  Keep the pre-pass minimal so the first
                # exp fires early, and level the rest into the pass loops.
                emit_qT(0)
                emit_kt(0, 0)
                prep_at = {}
                for l2 in range(NJ):
                    prep_at.setdefault((0, max(0, l2 - 2)), []).append(("v", l2, None))
                for s in range(1, 4):
                    prep_at.setdefault((0, 4 * s - 3), []).append(("kt", 0, s))
                for c in range(1, 4):
                    prep_at.setdefault((c - 1, 12), []).append(("qt", c, None))
                    prep_at.setdefault((c - 1, 13), []).append(("kt", c, 0))
                    for s in range(1, 4):
                        prep_at.setdefault((c, 4 * s - 3), []).append(("kt", c, s))

                # softmax division: Z-row cast to SBUF (DVE), PE outer-product
                # broadcast, fast-reciprocal + multiply (DVE).  The PE ops of
                # pass p's division are deferred until pass p+1's iter 1 so
                # the in-order PE queue never stalls on the Z cast.
                def div_stage1(p, ctx_pair):
                    zbf = wp.tile([1, 2 * SH], BF16, tag="zbf", name=f"zbf{p}")
                    nc.vector.tensor_copy(zbf, ctx_pair[HD:HD + 1, :])
                    return zbf

                def div_stage2(p, ctx_pair, zbf, pool, pool_tag):
                    bc = pool.tile([64, 2 * SH], F32, tag=pool_tag, name=f"bc{p}")
                    for t in range(2):
                        nc.tensor.matmul(
                            bc[:, 512 * t:512 * t + 512],
                            ones_bf[0:1, 0:HD],
                            zbf[0:1, 512 * t:512 * t + 512],
                            start=True, stop=True,
                        )
                    rb = wp.tile([64, 2 * SH], F32, tag="rb", name=f"rb{p}")
                    nc.vector.reciprocal_approx_fast(rb, bc)
                    for t in range(2):
                        nc.vector.tensor_tensor(
                            out=ctxTs_sb[p][64 * t:64 * t + 64, :],
                            in0=ctx_pair[0:HD, 512 * t:512 * t + 512],
                            in1=rb[:, 512 * t:512 * t + 512],
                            op=OP.mult,
                        )

                pending_div = None
                # Keep the sc-slot A/B alternation intact: prep/bc tiles
                # share the "sc" tag rotation, and an ODD number of them
                # between consecutive score tiles makes the next scores
                # land on the just-read slot (a ~2us WAR stall on exp).
                # A zero-cost dummy allocation restores parity.
                alloc_ct = [0]

                for p in range(4):
                    ctx_pair = cxp.tile([HD + 1, 2 * SH], F32, tag="ctx", name=f"ctx{p}")
                    for j in range(NJ):
                        sc = scp.tile([128, 2 * SH], F32, tag="sc")
                        for t in range(2):
                            nc.tensor.matmul(
                                sc[:, SH * t:SH * t + SH],
                                kt_sb[p][64 * t:64 * t + 64, 128 * j:128 * j + 128],
                                qT_sb[p][64 * t:64 * t + 64, :],
                                start=True, stop=True,
                            )
                        e = ep.tile([128, 2 * SH], BF16, tag="exp")
                        nc.scalar.activation(e, sc, AF.Exp)
                        if j in MASK_SLOTS:
                            slot = MASK_SLOTS.index(j)
                            c0, c1 = STRIPS[slot]
                            for t in range(2):
                                mask_engine.tensor_tensor(
                                    out=e[:, SH * t + c0:SH * t + c1],
                                    in0=e[:, SH * t + c0:SH * t + c1],
                                    in1=mask_sb[slot],
                                    op=OP.mult,
                                )
                        # prep matmuls AFTER this iter's scores: they fill
                        # the PE while exp runs instead of delaying it
                        for kind, a, b in prep_at.get((p, j), []):
                            if kind == "kt":
                                emit_kt(a, b)
                            elif kind == "qt":
                                emit_qT(a)
                            else:
                                emit_v(a)
                            alloc_ct[0] += 1
                        for t in range(2):
                            h = 2 * p + t
                            nc.tensor.matmul(
                                ctx_pair[:, SH * t:SH * t + SH],
                                v_sb[j][:, (HD + 1) * h:(HD + 1) * h + HD + 1],
                                e[:, SH * t:SH * t + SH],
                                start=(j == 0), stop=(j == NJ - 1),
                            )
                        if j == 1 and pending_div is not None:
                            div_stage2(*pending_div, scp, "sc")
                            alloc_ct[0] += 1
                            pending_div = None
                        if alloc_ct[0] % 2 == 1:
                            scp.tile([128, 8], F32, tag="sc", name=f"par{p}_{j}")
                        alloc_ct[0] = 0
                    zbf = div_stage1(p, ctx_pair)
                    if p < 3:
                        pending_div = (p, ctx_pair, zbf)
                    else:
                        last_div = (p, ctx_pair, zbf)

              # ---- out_proj + residual + LayerNorm per query tile ----
              # (scp is closed here; cxp stays open so pass 3's division can
              # finish while the p=0..2 out_proj partials run on freed scp
              # banks.)  The residual rides the PSUM accumulation (identity
              # matmul on x_nat); the final (y - mu) * rstd runs on the
              # scalar engine (Identity activation with per-row scale/bias).
              with tc.tile_pool(name="ops", bufs=1, space="PSUM") as ops:
                po_t = [ops.tile([128, D], F32, tag=f"po{qt}", name=f"po{qt}") for qt in range(4)]
                for qt in range(4):
                    nc.tensor.matmul(
                        po_t[qt], ident_sb, x_nat_sb[qt],
                        start=True, stop=False,
                    )
                for pp_i in range(3):
                    for qt in range(4):
                        nc.tensor.matmul(
                            po_t[qt],
                            ctxTs_sb[pp_i][:, 128 * qt:128 * qt + 128],
                            woutT_sb[pp_i],
                            start=False, stop=False,
                        )
                div_stage2(*last_div, cxp, "ctx")
                for qt in range(4):
                    po = po_t[qt]
                    nc.tensor.matmul(
                        po,
                        ctxTs_sb[3][:, 128 * qt:128 * qt + 128],
                        woutT_sb[3],
                        start=False, stop=True,
                    )
                    stats = wp.tile([128, 6], F32, tag="stats")
                    nc.vector.bn_stats(stats, po)
                    mv = wp.tile([128, 2], F32, tag="mv")
                    nc.vector.bn_aggr(mv, stats)
                    veps = wp.tile([128, 1], F32, tag="veps")
                    nc.vector.tensor_scalar_add(veps, mv[:, 1:2], eps_t)
                    rvar = wp.tile([128, 1], F32, tag="rvar")
                    nc.vector.reciprocal_approx_fast(rvar, veps)
                    rstd = wp.tile([128, 1], F32, tag="rstd")
                    nc.scalar.activation(rstd, rvar, AF.Sqrt)
                    nbias = wp.tile([128, 1], F32, tag="nbias")
                    nc.vector.tensor_scalar(
                        out=nbias, in0=mv[:, 0:1], scalar1=rstd, scalar2=-1.0,
                        op0=OP.mult, op1=OP.mult,
                    )
                    t1 = wp.tile([128, D], F32, tag="t1")
                    nc.scalar.activation(t1, po, AF.Identity, bias=nbias, scale=rstd)
                    if not ln_trivial:
                        nc.vector.tensor_tensor(out=t1, in0=t1, in1=gamma_sb, op=OP.mult)
                        nc.vector.tensor_tensor(out=t1, in0=t1, in1=beta_sb, op=OP.add)
                    nc.sync.dma_start(out=out_d[128 * qt:128 * qt + 128, :], in_=t1)

    nc.compile()
    return nc


def _host_prep(x, in_proj_w, in_proj_b, out_proj_w, out_proj_b, ln_gamma, ln_beta, window_size):
    x = np.ascontiguousarray(np.asarray(x, dtype=np.float32))
    in_proj_w = np.asarray(in_proj_w, dtype=np.float32)
    in_proj_b = np.asarray(in_proj_b, dtype=np.float32)
    out_proj_w = np.asarray(out_proj_w, dtype=np.float32)
    out_proj_b = np.asarray(out_proj_b, dtype=np.float32)
    ln_gamma = np.asarray(ln_gamma, dtype=np.float32)
    ln_beta = np.asarray(ln_beta, dtype=np.float32)
    w = int(np.asarray(window_size))
    half = w // 2
    assert half <= 128, "mask slots only cover |k-q| <= 128"

    bf16 = ml_dtypes.bfloat16

    def dpack(a):  # [4*128, C] -> [128, 4*C] partition-major d-chunk packing
        r, cc = a.shape
        return np.ascontiguousarray(
            a.reshape(4, 128, cc).transpose(1, 0, 2).reshape(128, 4 * cc)
        )

    scale = np.float32(1.0 / np.sqrt(HD))
    W = in_proj_w.copy()
    W[0:D] *= scale
    winT = dpack(np.ascontiguousarray(W.T)).astype(bf16)            # [128, 4*3D]
    woutT = dpack(np.ascontiguousarray(out_proj_w.T)).astype(bf16)  # [128, 4*D]
    bq = np.ascontiguousarray((in_proj_b[0:D] * scale).reshape(4, 128).T)  # [128, 4]
    bout = (out_proj_b + out_proj_w @ in_proj_b[2 * D:3 * D]).reshape(1, D)
    gamma_b = np.ascontiguousarray(np.broadcast_to(ln_gamma, (128, D)))
    beta_b = np.ascontiguousarray(np.broadcast_to(ln_beta, (128, D)))
    strips = []
    for j in MASK_SLOTS[:-1]:
        c0 = max(0, 128 * j - half + 1)
        c1 = min(SH, 128 * j + 128 + half)
        strips.append((c0, max(c1, c0 + 1)))
    strips.append((0, max(1, min(SH, half))))

    in_maps = []
    for c in range(8):
        b, s = divmod(c, 4)
        rot = (SH * s + np.arange(L)) % L
        xT_rot = dpack(np.ascontiguousarray(x[b][rot].T)).astype(bf16)  # [128, 4*L]
        x_nat = dpack(np.ascontiguousarray(
            x[b][SH * s:SH * s + SH] + bout[None, 0, :]
        )).astype(bf16)  # [128, 4*D]
        q_true = SH * s + np.arange(SH)[None, :]
        mask_cols = []
        for i, j in enumerate(MASK_SLOTS):
            k_true = (SH * s + 128 * j + np.arange(128)[:, None]) % L
            dd = k_true - q_true
            banned = (dd >= -half) & (dd < half)
            c0, c1 = strips[i]
            mask_cols.append((1.0 - banned[:, c0:c1].astype(np.float32)))
        maskp = np.concatenate(mask_cols, axis=1)
        m = {
            "xTp": xT_rot, "xnatp": x_nat, "winTp": winT, "woutTp": woutT,
            "bq": bq, "maskp": maskp.astype(bf16),
            "ident": np.eye(128, dtype=np.float32).astype(bf16),
        }
        if not LN_TRIVIAL:
            m["gamma"] = gamma_b
            m["beta"] = beta_b
        in_maps.append(m)
    return in_maps


def kernel(x, in_proj_w, in_proj_b, out_proj_w, out_proj_b, ln_gamma, ln_beta, window_size):
    global _COMPILED, LAST_RESULT
    half = int(np.asarray(window_size)) // 2
    ln_trivial = bool(np.all(np.asarray(ln_gamma) == 1.0) and np.all(np.asarray(ln_beta) == 0.0))
    key = (half, ln_trivial)
    if _COMPILED is None or _COMPILED[0] != key:
        _COMPILED = (key, _build(half, ln_trivial))
    in_maps = _host_prep(x, in_proj_w, in_proj_b, out_proj_w, out_proj_b,
                         ln_gamma, ln_beta, window_size)
    res = run_bass_kernel_spmd(_COMPILED[1], in_maps, core_ids=list(range(8)))
    LAST_RESULT = res
    out = np.empty((B, L, D), np.float32)
    for c in range(8):
        b, s = divmod(c, 4)
        out[b, SH * s:SH * s + SH] = res.results[c]["out"]
    return out
